# revision 1
# baseline (speedup 1.0000x reference)
"""NeighborCorrelator Trainium2 kernel.

Math: xn = x/||x||_C, yn = y/||y||_C (per-pixel channel L2 norm, clamped at
1e-12); out[b, o=(i,j), h, w] = sum_c xn[b,c,h,w] * ynp[b,c,h+i,w+j] where
ynp is yn zero-padded by 3 on each spatial side. K=7 -> 49 offsets.
Shapes: x,y [4, 256, 256, 256] f32 -> out [4, 49, 256, 256] f32.

Strategy (8 NeuronCores, data-parallel over (batch, H-half)):
  - Each core gets x shard [256, 128, 256] and a zero-padded y halo slab
    [256, 134, 262] (H halo of 3 + W pad of 3, materialized on host).
  - On device: cast to bf16; for each 8x16 pixel patch, TensorE computes the
    cross-correlation band  psum[m=patch pixel, n=(14x22 y-window col)] =
    sum_c x[c, m] * y[c, n]  (C=256 as two K=128 PSUM-accumulated matmuls).
    The 49 useful offsets per pixel live at sheared positions
    n = (dh+i)*22 + (dw+j) of the 308-wide band.
  - Per-pixel sum-of-squares of x and y via ones-matmuls (M=1) on squared
    tiles; shipped to host as f32 maps.
  - Host: gathers the sheared stencil out of the bf16 bands and multiplies by
    rsqrt norm maps; assembles [4, 49, 256, 256].
"""
import os
import sys

sys.path.insert(0, '/opt/trn_rl_repo')

import numpy as np
import ml_dtypes

import concourse.bass as bass
import concourse.bacc as bacc
import concourse.tile as tile
from concourse import mybir
from concourse.bass_utils import run_bass_kernel_spmd

B, C, H, W = 4, 256, 256, 256
K = 7
PAD = K // 2
NCORES = 8
HL = H // 2            # 128 rows per core
YH, YW = HL + 2 * PAD, W + 2 * PAD   # 134, 262

# patch geometry
PH, PW = 8, 16         # stationary patch (M = 128 pixels)
WH, WW = PH + 2 * PAD, PW + 2 * PAD  # y window 14 x 22
NB = WH * WW           # band width 308
SLAB = 32              # h rows per slab
NSLAB = HL // SLAB     # 4
PTH, PTW = SLAB // PH, W // PW       # 4 x 16 patches per slab
NPATCH = NSLAB * PTH * PTW           # 256 per core
YSLAB = SLAB + 2 * PAD               # 38 y rows per slab
GRP = 8                # staging group rows

NTRIM = 176
BF16 = mybir.dt.bfloat16
F32 = mybir.dt.float32

_CACHED_NC = None


def _build():
    nc = bacc.Bacc("TRN2", target_bir_lowering=False)
    x_d = nc.dram_tensor("x", [C, NPATCH, 128], BF16, kind="ExternalInput")
    y_d = nc.dram_tensor("y", [C, YH, YW], BF16, kind="ExternalInput")
    bands_d = nc.dram_tensor("bands", [NPATCH, 4, 32, NTRIM], BF16, kind="ExternalOutput")
    ssx_d = nc.dram_tensor("ssx", [HL, W], F32, kind="ExternalOutput")
    ssy_d = nc.dram_tensor("ssy", [YH, YW], F32, kind="ExternalOutput")

    with tile.TileContext(nc) as tc:
        with tc.tile_pool(name="xslab", bufs=2) as xslabp, \
             tc.tile_pool(name="yslab", bufs=2) as yslabp, \
             tc.tile_pool(name="sq", bufs=2) as sqp, \
             tc.tile_pool(name="bandst", bufs=2) as bandp, \
             tc.tile_pool(name="rows", bufs=2) as rowp, \
             tc.tile_pool(name="consts", bufs=1) as constp, \
             tc.tile_pool(name="ps", bufs=6, space="PSUM") as psp, \
             tc.tile_pool(name="psn", bufs=2, space="PSUM") as psnp:

            ones_sb = constp.tile([128, 1], BF16)
            nc.vector.memset(ones_sb, 1.0)

            pass
            pass

            for s in range(NSLAB):
                x16 = xslabp.tile([128, 2, PTH * PTW * 128], BF16, tag="x16")
                y16 = yslabp.tile([128, 2, YSLAB, YW], BF16, tag="y16")
                h0 = s * SLAB
                yr0 = s * SLAB  # y halo rows [yr0, yr0+38)

                # ---- load + cast x slab (groups of 8 rows) ----
                for g in range(SLAB // GRP):
                    # g == pt_h (GRP == PH): x arrives bf16 patch-major
                    p0g = (s * PTH + g) * PTW
                    src = bass.AP(
                        tensor=x_d, offset=p0g * 128,
                        ap=[[NPATCH * 128, 128], [128 * NPATCH * 128, 2],
                            [1, PTW * 128]])
                    nc.sync.dma_start(
                        out=x16[:, :, g * PTW * 128:(g + 1) * PTW * 128], in_=src)
                    # squares for x norms (ACT) -> bf16 (patch-major order)
                    sq = sqp.tile([128, 2, GRP * W], BF16, tag="sq")
                    nc.scalar.activation(
                        out=sq, in_=x16[:, :, g * PTW * 128:(g + 1) * PTW * 128],
                        func=mybir.ActivationFunctionType.Square)
                    # ones-matmuls: sumsq over C for these 2048 pixels
                    row = rowp.tile([1, GRP * W], F32, tag="row")
                    for q in range(GRP * W // 512):
                        psn = psnp.tile([1, 512], F32, tag="psn")
                        for ch in range(2):
                            nc.tensor.matmul(
                                psn, ones_sb,
                                sq[:, ch, q * 512:(q + 1) * 512],
                                start=(ch == 0), stop=(ch == 1))
                        if q % 2 == 0:
                            nc.vector.tensor_copy(out=row[:, q * 512:(q + 1) * 512], in_=psn)
                        else:
                            nc.scalar.copy(out=row[:, q * 512:(q + 1) * 512], in_=psn)
                    # row is in patch-major pixel order; scatter to (h, w)
                    dst = bass.AP(
                        tensor=ssx_d, offset=(h0 + g * GRP) * W,
                        ap=[[PW, PTW], [W, PH], [1, PW]])
                    nc.sync.dma_start(out=dst, in_=row)

                # ---- load + cast y slab (groups of <=8 rows) ----
                ngr = (YSLAB + GRP - 1) // GRP
                for g in range(ngr):
                    r0 = g * GRP
                    nr = min(GRP, YSLAB - r0)
                    src = bass.AP(
                        tensor=y_d, offset=(yr0 + r0) * YW,
                        ap=[[YH * YW, 128], [128 * YH * YW, 2], [1, nr * YW]])
                    nc.sync.dma_start(out=y16[:, :, r0:r0 + nr, :], in_=src)
                    # y norms: only rows not already covered by previous slab
                    nskip = 0 if s == 0 else (2 * PAD if g == 0 else 0)
                    a0 = r0 + nskip
                    na = nr - nskip
                    if na <= 0:
                        continue
                    sq = sqp.tile([128, 2, GRP, YW], BF16, tag="sq")
                    nc.vector.tensor_mul(
                        out=sq[:, :, :na, :], in0=y16[:, :, a0:a0 + na, :],
                        in1=y16[:, :, a0:a0 + na, :])
                    row = rowp.tile([1, GRP * YW], F32, tag="row")
                    npix = na * YW
                    nq = (npix + 511) // 512
                    for q in range(nq):
                        w0 = q * 512
                        w1 = min(npix, w0 + 512)
                        psn = psnp.tile([1, 512], F32, tag="psn")
                        for ch in range(2):
                            nc.tensor.matmul(
                                psn[:, :w1 - w0], ones_sb,
                                sq[:, ch, :, :].rearrange("p a b -> p (a b)")[:, w0:w1],
                                start=(ch == 0), stop=(ch == 1))
                        if q % 2 == 0:
                            nc.vector.tensor_copy(out=row[:, w0:w1], in_=psn[:, :w1 - w0])
                        else:
                            nc.scalar.copy(out=row[:, w0:w1], in_=psn[:, :w1 - w0])
                    nc.sync.dma_start(
                        out=ssy_d[yr0 + a0:yr0 + a0 + na, :],
                        in_=row[:, :npix])

                # ---- correlation bands ----
                for ph in range(PTH):
                    bst = bandp.tile([128, PTW, NB], BF16, tag="bst")
                    eng = nc.vector if ph % 2 == 0 else nc.scalar
                    for pw in range(PTW):
                        ps = psp.tile([128, NB], F32, tag="band")
                        for ch in range(2):
                            lhsT = x16[:, ch, (ph * PTW + pw) * 128:(ph * PTW + pw) * 128 + 128]
                            rhs = bass.AP(
                                tensor=y16.tensor,
                                offset=y16.offset + ch * YSLAB * YW + (ph * PH) * YW + pw * PW,
                                ap=[[y16[:].ap[0][0], 128], [YW, WH], [1, WW]])
                            nc.tensor.matmul(ps, lhsT, rhs,
                                             start=(ch == 0), stop=(ch == 1))
                        if eng is nc.vector:
                            eng.tensor_copy(out=bst[:, pw, :], in_=ps)
                        else:
                            eng.copy(out=bst[:, pw, :], in_=ps)
                    p0 = (s * PTH + ph) * PTW
                    # trimmed band write: quad q (partitions 32q..32q+32)
                    # only needs cols [44q, 44q+176)
                    bstpp = bst[:].ap[0][0]
                    for q in range(4):
                        srcq = bass.AP(
                            tensor=bst.tensor,
                            offset=bst.offset + 32 * q * bstpp + 44 * q,
                            ap=[[bstpp, 32], [NB, PTW], [1, NTRIM]],
                        )
                        dstq = bass.AP(
                            tensor=bands_d,
                            offset=p0 * 4 * 32 * NTRIM + q * 32 * NTRIM,
                            ap=[[NTRIM, 32], [4 * 32 * NTRIM, PTW], [1, NTRIM]])
                        nc.sync.dma_start(out=dstq, in_=srcq)

    nc.finalize()
    return nc


def _host_gather(bands, ssx, ssy):
    """bands [NPATCH,128,NB] bf16, ssx [HL*W] f32, ssy [YH*YW] f32
    -> out core shard [49, HL, W] f32"""
    rnx = 1.0 / np.maximum(np.sqrt(ssx.astype(np.float64)), 1e-12)
    rny = 1.0 / np.maximum(np.sqrt(ssy.astype(np.float64)), 1e-12)
    rnx = rnx.astype(np.float32).reshape(HL, W)
    rny = rny.astype(np.float32).reshape(YH, YW)

    dh = np.arange(PH)[:, None, None, None]
    dw = np.arange(PW)[None, :, None, None]
    ii = np.arange(K)[None, None, :, None]
    jj = np.arange(K)[None, None, None, :]
    m_idx = np.broadcast_to(dh * PW + dw, (PH, PW, K, K)).reshape(-1)
    n_idx = ((dh + ii) * WW + (dw + jj) - 44 * (dh // 2)).reshape(-1)

    bands = bands.reshape(NPATCH, 128, NTRIM)
    ext = bands[:, m_idx, n_idx].astype(np.float32)      # [NPATCH, PH*PW*49]
    ext = ext.reshape(NSLAB, PTH, PTW, PH, PW, K, K)
    # -> [K, K, NSLAB, PTH, PH, PTW, PW] -> [49, HL, W]
    ext = ext.transpose(5, 6, 0, 1, 3, 2, 4).reshape(K * K, HL, W)

    rny_win = np.lib.stride_tricks.sliding_window_view(rny, (HL, W))  # [7,7,HL,W]
    ext *= rnx[None]
    ext *= rny_win.reshape(K * K, HL, W)
    return ext


def kernel(x: np.ndarray, y: np.ndarray) -> np.ndarray:
    global _CACHED_NC
    if _CACHED_NC is None:
        _CACHED_NC = _build()
    nc = _CACHED_NC

    x = np.ascontiguousarray(x, dtype=np.float32)
    y = np.ascontiguousarray(y, dtype=np.float32)
    x16h = x.astype(ml_dtypes.bfloat16)
    yp = np.zeros((B, C, YH + H - HL, YW), dtype=ml_dtypes.bfloat16)
    yp[:, :, PAD:PAD + H, PAD:PAD + W] = y.astype(ml_dtypes.bfloat16)

    in_maps = []
    for core in range(NCORES):
        b, half = divmod(core, 2)
        xs = x16h[b, :, half * HL:(half + 1) * HL, :]
        xs = xs.reshape(C, NSLAB, PTH, PH, PTW, PW).transpose(0, 1, 2, 4, 3, 5)
        xs = np.ascontiguousarray(xs.reshape(C, NPATCH, 128))
        ys = np.ascontiguousarray(yp[b, :, half * HL:half * HL + YH, :])
        in_maps.append({"x": xs, "y": ys})

    trace = bool(os.environ.get("BASS_TRACE"))
    if trace:
        try:
            from ntff_hook import install as _ihook
            _ihook()
        except Exception:
            try:
                _install_ntff_hook_inline()
            except Exception as e:
                print(f"(ntff hook unavailable: {e})", file=sys.stderr)

    res = run_bass_kernel_spmd(nc, in_maps, core_ids=list(range(NCORES)),
                               trace=trace)
    if res.exec_time_ns:
        print(f"HW exec time: {res.exec_time_ns} ns")

    out = np.empty((B, K * K, H, W), dtype=np.float32)
    for core in range(NCORES):
        b, half = divmod(core, 2)
        r = res.results[core]
        bands = r["bands"].view(ml_dtypes.bfloat16) if r["bands"].dtype != ml_dtypes.bfloat16 else r["bands"]
        out[b, :, half * HL:(half + 1) * HL, :] = _host_gather(
            bands, r["ssx"].ravel(), r["ssy"].ravel())
    return out


def _install_ntff_hook_inline():
    import types
    import contextlib  # noqa
    mod = types.ModuleType("antenv.axon_hooks")
    _h = [None]
    mod.set_axon_ntff_profile_hook = lambda h: _h.__setitem__(0, h)
    mod.get_axon_ntff_profile_hook = lambda: _h[0]
    sys.modules["antenv.axon_hooks"] = mod
    import antenv
    antenv.axon_hooks = mod
    from trn_agent_boot.trn_boot import _ntff_profile_via_ctypes
    mod.set_axon_ntff_profile_hook(
        _ntff_profile_via_ctypes('/opt/axon/libaxon_pjrt.so'))


if __name__ == "__main__":
    rng = np.random.default_rng(0)
    xx = rng.standard_normal((B, C, H, W), dtype=np.float32)
    yy = rng.standard_normal((B, C, H, W), dtype=np.float32)
    o = kernel(x=xx, y=yy)
    print("out", o.shape, o.dtype)



# revision 2
# speedup vs baseline: 1.5289x; 1.5289x over previous
"""NeighborCorrelator Trainium2 kernel (v2).

Math: xn = x/||x||_C, yn = y/||y||_C (per-pixel channel L2 norm, clamped at
1e-12); out[b, o=(i,j), h, w] = sum_c xn[b,c,h,w] * ynp[b,c,h+i,w+j] where
ynp is yn zero-padded by 3 on each spatial side. K=7 -> 49 offsets.
Shapes: x,y [4, 256, 256, 256] f32 -> out [4, 49, 256, 256] f32.

Strategy (8 NeuronCores, data-parallel over (batch, H-half)):
  - Each core: x shard [256, 128, 256] bf16 patch-major, y halo slab
    [256, 134, 262] bf16 (H halo 3 + W pad 3, materialized on host).
  - Patch = 16x8 pixels (M=128); per patch TensorE computes the band
    psum[m, n=(22x14 window col)] = sum_c x[c,m] y[c,n] as two K=128
    PSUM-accumulated bf16 matmuls.  The 49 offsets per pixel live at
    sheared positions n = (dh+i)*14 + (dw+j).
  - Band copied PSUM->SBUF bf16 (ACT/DVE alternating), then trimmed to
    dh-pair blocks: partitions 16t..16t+16 only need cols [28t, 28t+112).
  - Host: sum-of-squares norms (f32), gather of the sheared stencil,
    multiply by rsqrt norm maps; assembles [4, 49, 256, 256].
"""
import os
import sys

sys.path.insert(0, '/opt/trn_rl_repo')

import numpy as np
import ml_dtypes

import concourse.bass as bass
import concourse.bacc as bacc
import concourse.tile as tile
from concourse import mybir
from concourse.bass_utils import run_bass_kernel_spmd

B, C, H, W = 4, 256, 256, 256
K = 7
PAD = K // 2
NCORES = 8
HL = H // 2                          # 128 rows per core
YH, YW = HL + 2 * PAD, W + 2 * PAD   # 134, 262

# patch geometry
PH, PW = 16, 8                       # stationary patch (M = 128 pixels)
WH, WW = PH + 2 * PAD, PW + 2 * PAD  # y window 22 x 14
NB = WH * WW                         # band width 308
SLAB = 32                            # h rows per slab
NSLAB = HL // SLAB                   # 4
PTH, PTW = SLAB // PH, W // PW       # 2 x 32 patches per slab
NPATCH = NSLAB * PTH * PTW           # 256 per core
YSLAB = SLAB + 2 * PAD               # 38 y rows per slab

NTP = PH // 2                        # 8 dh-pair blocks per patch
NTRIM = 112                          # cols kept per dh-pair block

BF16 = mybir.dt.bfloat16
F32 = mybir.dt.float32
EPS = 1e-12

_CACHED_NC = None


def _build():
    nc = bacc.Bacc("TRN2", target_bir_lowering=False)
    x_d = nc.dram_tensor("x", [C, NPATCH, 128], BF16, kind="ExternalInput")
    y_d = nc.dram_tensor("y", [C, YH, YW], BF16, kind="ExternalInput")
    bands_d = nc.dram_tensor("bands", [NPATCH, NTP, 16, NTRIM], BF16,
                             kind="ExternalOutput")

    with tile.TileContext(nc) as tc:
        with tc.tile_pool(name="xslab", bufs=2) as xp, \
             tc.tile_pool(name="yslab", bufs=2) as yp, \
             tc.tile_pool(name="bandst", bufs=2) as bandp, \
             tc.tile_pool(name="ps", bufs=8, space="PSUM") as psp:

            for s in range(NSLAB):
                y16 = yp.tile([128, 2, YSLAB, YW], BF16, tag="y16")
                ysrc = bass.AP(
                    tensor=y_d, offset=s * SLAB * YW,
                    ap=[[YH * YW, 128], [128 * YH * YW, 2], [1, YSLAB * YW]])
                nc.sync.dma_start(out=y16, in_=ysrc)
                ypp = y16[:].ap[0][0]

                for ph in range(PTH):
                    p0 = (s * PTH + ph) * PTW
                    x16 = xp.tile([128, 2, PTW * 128], BF16, tag="x16")
                    xsrc = bass.AP(
                        tensor=x_d, offset=p0 * 128,
                        ap=[[NPATCH * 128, 128], [128 * NPATCH * 128, 2],
                            [1, PTW * 128]])
                    nc.sync.dma_start(out=x16, in_=xsrc)

                    bst = bandp.tile([128, PTW, NB], BF16, tag="bst")
                    for pw in range(PTW):
                        ps = psp.tile([128, NB], F32, tag="ps")
                        for ch in range(2):
                            lhsT = x16[:, ch, pw * 128:(pw + 1) * 128]
                            rhs = bass.AP(
                                tensor=y16.tensor,
                                offset=(y16.offset + ch * YSLAB * YW
                                        + ph * PH * YW + pw * PW),
                                ap=[[ypp, 128], [YW, WH], [1, WW]])
                            nc.tensor.matmul(ps, lhsT, rhs,
                                             start=(ch == 0), stop=(ch == 1))
                        if pw % 2 == 0:
                            nc.vector.tensor_copy(out=bst[:, pw, :], in_=ps)
                        else:
                            nc.scalar.copy(out=bst[:, pw, :], in_=ps)

                    bstpp = bst[:].ap[0][0]
                    for t in range(NTP):
                        src = bass.AP(
                            tensor=bst.tensor,
                            offset=bst.offset + 16 * t * bstpp + 28 * t,
                            ap=[[bstpp, 16], [NB, PTW], [1, NTRIM]])
                        dst = bass.AP(
                            tensor=bands_d,
                            offset=p0 * NTP * 16 * NTRIM + t * 16 * NTRIM,
                            ap=[[NTRIM, 16], [NTP * 16 * NTRIM, PTW],
                                [1, NTRIM]])
                        nc.sync.dma_start(out=dst, in_=src)

    nc.finalize()
    return nc


# gather index arrays: pixel (dh, dw), offset (i, j)
_dh = np.arange(PH)[:, None, None, None]
_dw = np.arange(PW)[None, :, None, None]
_ii = np.arange(K)[None, None, :, None]
_jj = np.arange(K)[None, None, None, :]
_TP = np.broadcast_to(_dh // 2, (PH, PW, K, K)).reshape(-1)
_R16 = np.broadcast_to((_dh % 2) * 8 + _dw, (PH, PW, K, K)).reshape(-1)
_CC = (((_dh % 2) + _ii) * WW + _dw + _jj).reshape(-1)


def _host_gather(bands, rnx, rny):
    """bands [NPATCH, NTP, 16, NTRIM] bf16, rnx [HL, W] f32,
    rny [YH, YW] f32 -> core shard [49, HL, W] f32"""
    ext = bands[:, _TP, _R16, _CC].astype(np.float32)    # [NPATCH, 128*49]
    ext = ext.reshape(NSLAB, PTH, PTW, PH, PW, K, K)
    ext = ext.transpose(5, 6, 0, 1, 3, 2, 4).reshape(K * K, HL, W)
    rny_win = np.lib.stride_tricks.sliding_window_view(rny, (HL, W))
    ext *= rnx[None]
    ext *= rny_win.reshape(K * K, HL, W)
    return ext


def kernel(x: np.ndarray, y: np.ndarray) -> np.ndarray:
    global _CACHED_NC
    if _CACHED_NC is None:
        _CACHED_NC = _build()
    nc = _CACHED_NC

    x = np.ascontiguousarray(x, dtype=np.float32)
    y = np.ascontiguousarray(y, dtype=np.float32)
    x16h = x.astype(ml_dtypes.bfloat16)
    yp16 = np.zeros((B, C, H + 2 * PAD, W + 2 * PAD), dtype=ml_dtypes.bfloat16)
    yp16[:, :, PAD:PAD + H, PAD:PAD + W] = y.astype(ml_dtypes.bfloat16)

    # per-pixel channel sum-of-squares -> rsqrt maps (f32, host)
    rnx = 1.0 / np.maximum(np.sqrt((x * x).sum(axis=1)), EPS)      # [B, H, W]
    ssy = np.zeros((B, H + 2 * PAD, W + 2 * PAD), dtype=np.float32)
    ssy[:, PAD:PAD + H, PAD:PAD + W] = (y * y).sum(axis=1)
    rny = 1.0 / np.maximum(np.sqrt(ssy), EPS)                      # [B, 262, 262]

    in_maps = []
    for core in range(NCORES):
        b, half = divmod(core, 2)
        xs = x16h[b, :, half * HL:(half + 1) * HL, :]
        xs = xs.reshape(C, NSLAB, PTH, PH, PTW, PW).transpose(0, 1, 2, 4, 3, 5)
        xs = np.ascontiguousarray(xs.reshape(C, NPATCH, 128))
        ys = np.ascontiguousarray(yp16[b, :, half * HL:half * HL + YH, :])
        in_maps.append({"x": xs, "y": ys})

    trace = bool(os.environ.get("BASS_TRACE"))
    if trace:
        try:
            from ntff_hook import install as _ihook
            _ihook()
        except Exception:
            try:
                _install_ntff_hook_inline()
            except Exception as e:
                print(f"(ntff hook unavailable: {e})", file=sys.stderr)

    res = run_bass_kernel_spmd(nc, in_maps, core_ids=list(range(NCORES)),
                               trace=trace)
    if res.exec_time_ns:
        print(f"HW exec time: {res.exec_time_ns} ns")

    out = np.empty((B, K * K, H, W), dtype=np.float32)
    for core in range(NCORES):
        b, half = divmod(core, 2)
        r = res.results[core]
        bands = (r["bands"].view(ml_dtypes.bfloat16)
                 if r["bands"].dtype != ml_dtypes.bfloat16 else r["bands"])
        bands = bands.reshape(NPATCH, NTP, 16, NTRIM)
        out[b, :, half * HL:(half + 1) * HL, :] = _host_gather(
            bands,
            rnx[b, half * HL:(half + 1) * HL],
            rny[b, half * HL:half * HL + YH])
    return out


def _install_ntff_hook_inline():
    import types
    import contextlib  # noqa
    mod = types.ModuleType("antenv.axon_hooks")
    _h = [None]
    mod.set_axon_ntff_profile_hook = lambda h: _h.__setitem__(0, h)
    mod.get_axon_ntff_profile_hook = lambda: _h[0]
    sys.modules["antenv.axon_hooks"] = mod
    import antenv
    antenv.axon_hooks = mod
    from trn_agent_boot.trn_boot import _ntff_profile_via_ctypes
    mod.set_axon_ntff_profile_hook(
        _ntff_profile_via_ctypes('/opt/axon/libaxon_pjrt.so'))


if __name__ == "__main__":
    rng = np.random.default_rng(0)
    xx = rng.standard_normal((B, C, H, W), dtype=np.float32)
    yy = rng.standard_normal((B, C, H, W), dtype=np.float32)
    o = kernel(x=xx, y=yy)
    print("out", o.shape, o.dtype)


# revision 10
# speedup vs baseline: 1.6186x; 1.0587x over previous
"""NeighborCorrelator Trainium2 kernel (v3).

Math: xn = x/||x||_C, yn = y/||y||_C (per-pixel channel L2 norm, clamped at
1e-12); out[b, o=(i,j), h, w] = sum_c xn[b,c,h,w] * ynp[b,c,h+i,w+j] where
ynp is yn zero-padded by 3 on each spatial side. K=7 -> 49 offsets.
Shapes: x,y [4, 256, 256, 256] f32 -> out [4, 49, 256, 256] f32.

Strategy (8 NeuronCores, data-parallel over (batch, H-half)):
  - Each core: x shard [256, 128, 256] bf16 patch-major, y halo slab
    [256, 134, 262] bf16 (H halo 3 + W pad 3, materialized on host).
  - Patch = 16x8 pixels (M=128); per patch TensorE computes the band
    psum[m, n=(22x14 window col)] = sum_c x[c,m] y[c,n] as two K=128
    PSUM-accumulated bf16 matmuls.  The 49 offsets per pixel live at
    sheared positions n = (dh+i)*14 + (dw+j).
  - Loads are software-pipelined (next slab's y / next group's x issued
    before current group's compute) on the sync engine's HWDGE so no
    dma_start queues behind a blocked store.
  - Bands copied PSUM->SBUF bf16 two patches per instruction (ACT/DVE
    alternating), then written out with a single 4D-AP DMA per group,
    trimmed per dh-block: partitions 8*dh..8*dh+8 only need cols
    [14*dh, 14*dh+98).
  - Host: sum-of-squares norms (f32), gather of the sheared stencil,
    multiply by rsqrt norm maps; assembles [4, 49, 256, 256].
"""
import os
import sys

sys.path.insert(0, '/opt/trn_rl_repo')

import numpy as np
import ml_dtypes

import concourse.bass as bass
import concourse.bacc as bacc
import concourse.tile as tile
from concourse import mybir
from concourse.bass_utils import run_bass_kernel_spmd

B, C, H, W = 4, 256, 256, 256
K = 7
PAD = K // 2
NCORES = 8
HL = H // 2                          # 128 rows per core
YH, YW = HL + 2 * PAD, W + 2 * PAD   # 134, 262

# patch geometry
PH, PW = 16, 8                       # stationary patch (M = 128 pixels)
WH, WW = PH + 2 * PAD, PW + 2 * PAD  # y window 22 x 14
NB = WH * WW                         # band width 308
SLAB = 32                            # h rows per slab
NSLAB = HL // SLAB                   # 4
PTH, PTW = SLAB // PH, W // PW       # 2 x 32 patches per slab
NG = NSLAB * PTH                     # 8 patch groups per core
NPATCH = NG * PTW                    # 256 per core
YSLAB = SLAB + 2 * PAD               # 38 y rows per slab

NTP = PH // 2                        # 8 dh-pair blocks per patch
NTRIM = 112                          # cols kept per dh-pair block

BF16 = mybir.dt.bfloat16
F32 = mybir.dt.float32
EPS = 1e-12

_CACHED_NC = None


def _build():
    nc = bacc.Bacc("TRN2", target_bir_lowering=False)
    x_d = nc.dram_tensor("x", [C, NPATCH, 128], BF16, kind="ExternalInput")
    y_d = nc.dram_tensor("y", [C, YH, YW], BF16, kind="ExternalInput")
    bands_d = nc.dram_tensor("bands", [NPATCH, NTP, 16, NTRIM], BF16,
                             kind="ExternalOutput")

    with tile.TileContext(nc) as tc:
        with tc.tile_pool(name="xslab", bufs=2) as xp, \
             tc.tile_pool(name="yslab", bufs=3) as yp, \
             tc.tile_pool(name="bandst", bufs=2) as bandp, \
             tc.tile_pool(name="ps", bufs=4, space="PSUM") as psp:

            YRA = WH                     # rows 0..22 (enough for ph=0)
            def load_y(s, part):
                """part 0: alloc tile + load rows [0, 22); part 1: rows
                [22, 38) into the same tile."""
                if part == 0:
                    t = yp.tile([128, 2, YSLAB, YW], BF16, tag="y16")
                    r0, nr = 0, YRA
                else:
                    t, r0, nr = part[0], YRA, YSLAB - YRA
                src = bass.AP(
                    tensor=y_d, offset=(s * SLAB + r0) * YW,
                    ap=[[YH * YW, 128], [128 * YH * YW, 2], [1, nr * YW]])
                nc.sync.dma_start(out=t[:, :, r0:r0 + nr, :], in_=src)
                return t

            def load_x(g):
                t = xp.tile([128, 2, PTW * 128], BF16, tag="x16")
                src = bass.AP(
                    tensor=x_d, offset=g * PTW * 128,
                    ap=[[NPATCH * 128, 128], [128 * NPATCH * 128, 2],
                        [1, PTW * 128]])
                nc.sync.dma_start(out=t, in_=src)
                return t

            # prologue: x for group 0, then y slab 0 (split), x for group 1,
            # y slab 1
            xnext = load_x(0)
            ytiles = [None] * NSLAB
            ytiles[0] = load_y(0, 0)
            load_y(0, (ytiles[0],))
            ytiles[1] = load_y(1, 0)
            load_y(1, (ytiles[1],))
            for s in range(NSLAB):
                ycur = ytiles[s]
                ypp = ycur[:].ap[0][0]
                for ph in range(PTH):
                    g = s * PTH + ph
                    if ph == 0 and s + 2 < NSLAB:
                        ytiles[s + 2] = load_y(s + 2, 0)
                        load_y(s + 2, (ytiles[s + 2],))
                    x16 = xnext
                    xnext = load_x(g + 1) if g + 1 < NG else None

                    bst = bandp.tile([128, PTW, NB], BF16, tag="bst")
                    for pr in range(PTW // 2):
                        ps = psp.tile([128, 2, 512], F32, tag="ps")
                        for q in range(2):
                            pw = pr * 2 + q
                            for ch in range(2):
                                lhsT = x16[:, ch, pw * 128:(pw + 1) * 128]
                                rhs = bass.AP(
                                    tensor=ycur.tensor,
                                    offset=(ycur.offset + ch * YSLAB * YW
                                            + ph * PH * YW + pw * PW),
                                    ap=[[ypp, 128], [YW, WH], [1, WW]])
                                nc.tensor.matmul(ps[:, q, 0:NB], lhsT, rhs,
                                                 start=(ch == 0),
                                                 stop=(ch == 1))
                        if pr % 2 == 0:
                            nc.vector.tensor_copy(
                                out=bst[:, 2 * pr:2 * pr + 2, :],
                                in_=ps[:, :, 0:NB])
                        else:
                            nc.scalar.copy(
                                out=bst[:, 2 * pr:2 * pr + 2, :],
                                in_=ps[:, :, 0:NB])

                    # trimmed stores: dh-pair block t (partitions 16t..16t+16)
                    # keeps cols [28t, 28t+112)
                    bstpp = bst[:].ap[0][0]
                    for t in range(NTP):
                        src = bass.AP(
                            tensor=bst.tensor,
                            offset=bst.offset + 16 * t * bstpp + 28 * t,
                            ap=[[bstpp, 16], [NB, PTW], [1, NTRIM]])
                        dst = bass.AP(
                            tensor=bands_d,
                            offset=(g * PTW * NTP + t) * 16 * NTRIM,
                            ap=[[NTRIM, 16], [NTP * 16 * NTRIM, PTW],
                                [1, NTRIM]])
                        nc.sync.dma_start(out=dst, in_=src)

    nc.finalize()
    return nc


# gather index arrays: pixel (dh, dw), offset (i, j)
_dh = np.arange(PH)[:, None, None, None]
_dw = np.arange(PW)[None, :, None, None]
_ii = np.arange(K)[None, None, :, None]
_jj = np.arange(K)[None, None, None, :]
_TP = np.broadcast_to(_dh // 2, (PH, PW, K, K)).reshape(-1)
_R16 = np.broadcast_to((_dh % 2) * 8 + _dw, (PH, PW, K, K)).reshape(-1)
_CC = (((_dh % 2) + _ii) * WW + _dw + _jj).reshape(-1)


def _host_gather(bands, rnx, rny):
    """bands [NPATCH, NTP, 16, NTRIM] bf16, rnx [HL, W] f32,
    rny [YH, YW] f32 -> core shard [49, HL, W] f32"""
    ext = bands[:, _TP, _R16, _CC].astype(np.float32)   # [NPATCH, 128*49]
    ext = ext.reshape(NSLAB, PTH, PTW, PH, PW, K, K)
    ext = ext.transpose(5, 6, 0, 1, 3, 2, 4).reshape(K * K, HL, W)
    rny_win = np.lib.stride_tricks.sliding_window_view(rny, (HL, W))
    ext *= rnx[None]
    ext *= rny_win.reshape(K * K, HL, W)
    return ext


def kernel(x: np.ndarray, y: np.ndarray) -> np.ndarray:
    global _CACHED_NC
    if _CACHED_NC is None:
        _CACHED_NC = _build()
    nc = _CACHED_NC

    x = np.ascontiguousarray(x, dtype=np.float32)
    y = np.ascontiguousarray(y, dtype=np.float32)
    x16h = x.astype(ml_dtypes.bfloat16)
    yp16 = np.zeros((B, C, H + 2 * PAD, W + 2 * PAD), dtype=ml_dtypes.bfloat16)
    yp16[:, :, PAD:PAD + H, PAD:PAD + W] = y.astype(ml_dtypes.bfloat16)

    # per-pixel channel sum-of-squares -> rsqrt maps (f32, host)
    rnx = 1.0 / np.maximum(np.sqrt((x * x).sum(axis=1)), EPS)      # [B, H, W]
    ssy = np.zeros((B, H + 2 * PAD, W + 2 * PAD), dtype=np.float32)
    ssy[:, PAD:PAD + H, PAD:PAD + W] = (y * y).sum(axis=1)
    rny = 1.0 / np.maximum(np.sqrt(ssy), EPS)                      # [B, 262, 262]

    in_maps = []
    for core in range(NCORES):
        b, half = divmod(core, 2)
        xs = x16h[b, :, half * HL:(half + 1) * HL, :]
        xs = xs.reshape(C, NSLAB, PTH, PH, PTW, PW).transpose(0, 1, 2, 4, 3, 5)
        xs = np.ascontiguousarray(xs.reshape(C, NPATCH, 128))
        ys = np.ascontiguousarray(yp16[b, :, half * HL:half * HL + YH, :])
        in_maps.append({"x": xs, "y": ys})

    trace = bool(os.environ.get("BASS_TRACE"))
    if trace:
        try:
            from ntff_hook import install as _ihook
            _ihook()
        except Exception:
            try:
                _install_ntff_hook_inline()
            except Exception as e:
                print(f"(ntff hook unavailable: {e})", file=sys.stderr)

    res = run_bass_kernel_spmd(nc, in_maps, core_ids=list(range(NCORES)),
                               trace=trace)
    if res.exec_time_ns:
        print(f"HW exec time: {res.exec_time_ns} ns")

    out = np.empty((B, K * K, H, W), dtype=np.float32)
    for core in range(NCORES):
        b, half = divmod(core, 2)
        r = res.results[core]
        bands = (r["bands"].view(ml_dtypes.bfloat16)
                 if r["bands"].dtype != ml_dtypes.bfloat16 else r["bands"])
        bands = bands.reshape(NPATCH, NTP, 16, NTRIM)
        out[b, :, half * HL:(half + 1) * HL, :] = _host_gather(
            bands,
            rnx[b, half * HL:(half + 1) * HL],
            rny[b, half * HL:half * HL + YH])
    return out


def _install_ntff_hook_inline():
    import types
    import contextlib  # noqa
    mod = types.ModuleType("antenv.axon_hooks")
    _h = [None]
    mod.set_axon_ntff_profile_hook = lambda h: _h.__setitem__(0, h)
    mod.get_axon_ntff_profile_hook = lambda: _h[0]
    sys.modules["antenv.axon_hooks"] = mod
    import antenv
    antenv.axon_hooks = mod
    from trn_agent_boot.trn_boot import _ntff_profile_via_ctypes
    mod.set_axon_ntff_profile_hook(
        _ntff_profile_via_ctypes('/opt/axon/libaxon_pjrt.so'))


if __name__ == "__main__":
    rng = np.random.default_rng(0)
    xx = rng.standard_normal((B, C, H, W), dtype=np.float32)
    yy = rng.standard_normal((B, C, H, W), dtype=np.float32)
    o = kernel(x=xx, y=yy)
    print("out", o.shape, o.dtype)
